# revision 27
# baseline (speedup 1.0000x reference)
"""ConvGRU (nn_ConvRNN) Trainium2 Bass kernel — 8-core SPMD, fp16 matmul path.

Sharding: 8 cores = (batch n in 0..3) x (H half in {top, bottom}). Each core
owns a 32-row band of one image for the whole pipeline:
  Phase 1: 5x5 conv 64->384 for all T=8 timesteps on the local band, conv
           output kept resident in SBUF (fp16), per-channel sum (ACT-copy
           accum) / sumsq (DVE tensor_tensor_reduce) partials, one 8-core
           AllGather for the global BN statistics.
  Phase 2: the GRU recurrence. Per step: a 2-row halo AllGather between
           H-half pairs (overlapped with interior conv via instruction
           order), 3x3 convs (zr: 256ch, hh: 128ch), per-conv BN-stat
           AllGather, fused BN+relu6+BN_y+sigmoid/tanh chains using the
           identity  relu6(BN(c)) + BN_y(y) = s1*(min(relu(BN(c))/s1,6/s1)
           + y) + t1  so raw y needs no separate affine pass.

All matmuls run in fp16 (1 cyc/row vs 4 for fp32); PSUM accumulation is
fp32; BN statistics are fp32. Conv biases are dropped: BN subtracts the
batch mean, so a per-channel bias added before BN cancels exactly.
"""

import os
import numpy as np

MM_DT = os.environ.get("KERNEL_MM_DT", "float16")

T, N, CIN, H, W = 8, 4, 64, 64, 64
CD = 128
NCORES = 8
EPS = 1e-5

HP1, WP1 = 38, 68        # phase-1 padded input rows/cols per core
ROWS = 34                # conv output rows stored per core
OWN = 32                 # own rows
HPAD, WPAD = 36, 66      # phase-2 padded h tile
CNT1 = float(T * N * H * W)      # 131072, x2h BN count
CNT2 = float(N * H * W)          # 16384, recurrence BN count

_PROG = None


def _build_program():
    import concourse.bacc as bacc
    import concourse.bass as bass
    import concourse.tile as tile
    from concourse import mybir

    f32 = mybir.dt.float32
    f16 = getattr(mybir.dt, MM_DT)
    AF = mybir.ActivationFunctionType
    ALU = mybir.AluOpType
    AX = mybir.AxisListType
    PAIRS = [[2 * i, 2 * i + 1] for i in range(NCORES // 2)]
    ALL = [list(range(NCORES))]

    nc = bacc.Bacc("TRN2", target_bir_lowering=False, debug=False,
                   enable_asserts=False, num_devices=NCORES)

    x2d = nc.dram_tensor("x2", [T, 128, HP1, WP1], f16, kind="ExternalInput")
    x3d = nc.dram_tensor("x3", [T, 128, HP1, 64], f16, kind="ExternalInput")
    wpd = nc.dram_tensor("wp", [128, 30, 128], f16, kind="ExternalInput")
    wqd = nc.dram_tensor("wq", [128, 6, 128], f16, kind="ExternalInput")
    wsd = nc.dram_tensor("ws", [64, 3, 128], f16, kind="ExternalInput")
    wzrd = nc.dram_tensor("wzr", [128, 18, 128], f16, kind="ExternalInput")
    whhd = nc.dram_tensor("whh", [128, 9, 128], f16, kind="ExternalInput")
    gxd = nc.dram_tensor("gx", [128, 3], f32, kind="ExternalInput")
    btxd = nc.dram_tensor("btx", [128, 3], f32, kind="ExternalInput")
    gzrd = nc.dram_tensor("gzr", [128, 2], f32, kind="ExternalInput")
    btzrd = nc.dram_tensor("btzr", [128, 2], f32, kind="ExternalInput")
    ghhd = nc.dram_tensor("ghh", [128, 1], f32, kind="ExternalInput")
    bthhd = nc.dram_tensor("bthh", [128, 1], f32, kind="ExternalInput")
    mpred = nc.dram_tensor("mpre", [128, 1], f32, kind="ExternalInput")
    mpostd = nc.dram_tensor("mpost", [128, 1], f32, kind="ExternalInput")
    outd = nc.dram_tensor("out", [T, 128, OWN, 64], f16, kind="ExternalOutput")

    def ag(ins_ap, outs_ap, groups):
        nc.gpsimd.collective_compute(
            "AllGather", ALU.bypass, replica_groups=groups,
            ins=[ins_ap], outs=[outs_ap])

    import concourse.bass as _b

    with tile.TileContext(nc) as tc:
        with tc.tile_pool(name="consts", bufs=1) as consts, \
             tc.tile_pool(name="ytiles", bufs=1) as ytiles, \
             tc.tile_pool(name="dram", bufs=1, space="DRAM") as dram, \
             tc.tile_pool(name="dram2", bufs=2, space="DRAM") as dram2, \
             tc.tile_pool(name="stp", bufs=2) as stp, \
             tc.tile_pool(name="scrp", bufs=2) as scrp, \
             tc.tile_pool(name="ps", bufs=3, space="PSUM") as ps, \
             tc.tile_pool(name="psE", bufs=1, space="PSUM") as psE, \
             tc.tile_pool(name="psD", bufs=1, space="PSUM") as psD:

            # ---- persistent weights / consts ----
            wzr_sb = consts.tile([128, 18, 128], f16)
            nc.sync.dma_start(wzr_sb, wzrd[:])
            whh_sb = consts.tile([128, 9, 128], f16)
            nc.sync.dma_start(whh_sb, whhd[:])
            gx_sb = consts.tile([128, 3], f32)
            nc.sync.dma_start(gx_sb, gxd[:])
            btx_sb = consts.tile([128, 3], f32)
            nc.sync.dma_start(btx_sb, btxd[:])
            gzr_sb = consts.tile([128, 2], f32)
            nc.sync.dma_start(gzr_sb, gzrd[:])
            btzr_sb = consts.tile([128, 2], f32)
            nc.sync.dma_start(btzr_sb, btzrd[:])
            ghh_sb = consts.tile([128, 1], f32)
            nc.sync.dma_start(ghh_sb, ghhd[:])
            bthh_sb = consts.tile([128, 1], f32)
            nc.sync.dma_start(bthh_sb, bthhd[:])
            mpre_sb = consts.tile([128, 1], f32)
            nc.sync.dma_start(mpre_sb, mpred[:])
            mpost_sb = consts.tile([128, 1], f32)
            nc.sync.dma_start(mpost_sb, mpostd[:])

            eps_sb = consts.tile([128, 1], f32)
            nc.vector.memset(eps_sb, EPS)
            h_pad = consts.tile([128, HPAD, WPAD], f16)
            nc.vector.memset(h_pad, 0.0)
            rh_pad = consts.tile([128, HPAD, WPAD], f16)
            nc.vector.memset(rh_pad, 0.0)
            # warm-keeper resources: dependency-free matmuls + ACT-table
            # preloads that run while PE/ACT would otherwise idle at a
            # collective barrier (keeps the HAM clock at 2.4 GHz and the
            # activation table hot)
            xdum = consts.tile([128, 8, 64], f16)
            nc.vector.memset(xdum, 0.0)
            wdum = consts.tile([128, 128], f16)
            nc.vector.memset(wdum, 0.0)
            jnk_in = consts.tile([128, 1], f32)
            nc.vector.memset(jnk_in, 1.0)
            jnk_out = consts.tile([128, 1], f32)
            psdum = psD.tile([128, 8, 64], f32)

            def warm(n):
                for _ in range(n):
                    nc.tensor.matmul(psdum, wdum, xdum,
                                     start=True, stop=True)

            def preload(fns):
                for fn in fns:
                    nc.scalar.activation(jnk_out, jnk_in, fn)

            # y conv outputs, SBUF-resident for the whole kernel
            y_sb = []
            for t in range(T):
                row = []
                for c3 in range(3):
                    yt = ytiles.tile([128, ROWS, 64], f16,
                                     tag=f"y{t}_{c3}", name=f"y{t}_{c3}")
                    row.append(yt)
                y_sb.append(row)

            st1sum = consts.tile([128, 3, T, 2], f32)
            st1sq = consts.tile([128, 3, T], f32)
            scale1 = consts.tile([128, 3], f32)
            shift1 = consts.tile([128, 3], f32)
            inv1 = consts.tile([128, 3], f32)
            inv6 = consts.tile([128, 3], f32)
            gi_zr = consts.tile([128, 2], f32)
            btinv_zr = consts.tile([128, 2], f32)
            gi_hh = consts.tile([128, 1], f32)
            btinv_hh = consts.tile([128, 1], f32)

            # ================= Phase 1: 5x5 conv 64->384 =================
            with tc.tile_pool(name="w1", bufs=1) as w1, \
                 tc.tile_pool(name="x1", bufs=2) as x1:
                wp_sb = w1.tile([128, 30, 128], f16)
                nc.sync.dma_start(wp_sb, wpd[:])
                wq_sb = w1.tile([128, 6, 128], f16)
                nc.sync.dma_start(wq_sb, wqd[:])
                ws_sb = w1.tile([64, 3, 128], f16)
                nc.sync.dma_start(ws_sb, wsd[:])

                for t in range(T):
                    x2t = x1.tile([128, HP1, WP1], f16, tag="x2t")
                    nc.sync.dma_start(x2t[:, 0:19, :], x2d[t, :, 0:19, :])
                    nc.sync.dma_start(x2t[:, 19:38, :], x2d[t, :, 19:38, :])
                    x3t = x1.tile([128, HP1, 64], f16, tag="x3t")
                    nc.sync.dma_start(x3t[:, 0:19, :], x3d[t, :, 0:19, :])
                    nc.sync.dma_start(x3t[:, 19:38, :], x3d[t, :, 19:38, :])

                    for c3 in range(3):
                        # weight-stationary: each tap's weight drives all 4
                        # strips (+ edge rows) before the next LDWEIGHTS
                        taps = []
                        for ky in range(5):
                            for e in range(2):
                                taps.append((
                                    wp_sb[:, c3 * 10 + ky * 2 + e, :],
                                    lambda i0, ky=ky, e=e: x2t[
                                        :, i0 + ky:i0 + ky + 8,
                                        2 * e:2 * e + 64],
                                    x2t[:, ky::33, 2 * e:2 * e + 64]))
                        for p in range(2):
                            taps.append((
                                wq_sb[:, c3 * 2 + p, :],
                                lambda i0, p=p: x3t[
                                    :, i0 + 2 * p:i0 + 2 * p + 8, :],
                                x3t[:, 2 * p::33, :]))
                        taps.append((
                            ws_sb[:, c3, :],
                            lambda i0: x2t[0:64, i0 + 4:i0 + 12, 4:68],
                            x2t[0:64, 4::33, 4:68]))

                        pga = ps.tile([128, 16, 64], f32, tag="ps")
                        pgb = ps.tile([128, 16, 64], f32, tag="ps")
                        regions = [(pga, 0, 1), (pga, 1, 9),
                                   (pgb, 0, 17), (pgb, 1, 25)]
                        pse = None
                        if c3 == 1:
                            pse = psE.tile([128, 2, 64], f32, tag="pse",
                                           name="pse1")
                        nk = len(taps)
                        for k, (wap, mov, move) in enumerate(taps):
                            for (pg, si, i0) in regions:
                                nc.tensor.matmul(
                                    pg[:, 8 * si:8 * si + 8, :], wap,
                                    mov(i0), start=(k == 0),
                                    stop=(k == nk - 1))
                            if pse is not None:
                                nc.tensor.matmul(
                                    pse, wap, move, start=(k == 0),
                                    stop=(k == nk - 1))
                        nc.scalar.activation(
                            y_sb[t][c3][:, 1:17, :], pga, AF.Copy,
                            accum_out=st1sum[:, c3, t, 0:1])
                        nc.scalar.activation(
                            y_sb[t][c3][:, 17:33, :], pgb, AF.Copy,
                            accum_out=st1sum[:, c3, t, 1:2])
                        if pse is not None:
                            nc.scalar.activation(
                                y_sb[t][c3][:, 0::33, :], pse, AF.Copy)
                        scr = scrp.tile([128, OWN, 64], f16, tag="scr")
                        nc.vector.tensor_mul(scr, y_sb[t][c3][:, 1:33, :],
                                             y_sb[t][c3][:, 1:33, :])
                        nc.vector.tensor_scalar(
                            out=scr, in0=scr, scalar1=1.0, scalar2=None,
                            op0=ALU.mult, op1=ALU.add,
                            accum_out=st1sq[:, c3, t:t + 1])
                    if t == T - 2:
                        # early stats AllGather over t=0..6, hidden under
                        # the t=7 conv
                        stin1a = stp.tile([128, 3, 2], f32, tag="stin1a")
                        for c3 in range(3):
                            nc.vector.reduce_sum(
                                stin1a[:, c3, 0:1],
                                st1sum[:, c3, 0:T - 1, :], axis=AX.XY)
                            nc.vector.reduce_sum(
                                stin1a[:, c3, 1:2],
                                st1sq[:, c3, 0:T - 1], axis=AX.X)
                        ag1ia = dram.tile([128, 3, 2], f32)
                        ag1oa = dram.tile([NCORES, 128, 6], f32)
                        nc.sync.dma_start(ag1ia, stin1a)
                        ag(ag1ia.opt(), ag1oa.opt(), ALL)

            # ---- phase-1 BN stats: t=7 AllGather + combine with early AG ----
            stin1 = stp.tile([128, 3, 2], f32, tag="stin1")
            for c3 in range(3):
                nc.vector.reduce_sum(stin1[:, c3, 0:1],
                                     st1sum[:, c3, T - 1, :], axis=AX.X)
                nc.vector.tensor_copy(stin1[:, c3, 1:2],
                                      st1sq[:, c3, T - 1:T])
            ag1i = dram.tile([128, 3, 2], f32)
            ag1o = dram.tile([NCORES, 128, 6], f32)
            nc.sync.dma_start(ag1i, stin1)
            ag(ag1i.opt(), ag1o.opt(), ALL)
            warm(72)
            preload([AF.Sqrt])
            g1 = stp.tile([128, 6, NCORES, 2], f32, tag="g1")
            nc.sync.dma_start(
                g1[:, :, :, 0],
                _b.AP(tensor=ag1oa.tensor, offset=ag1oa.offset,
                      ap=[[6, 128], [1, 6], [768, NCORES]]))
            nc.sync.dma_start(
                g1[:, :, :, 1],
                _b.AP(tensor=ag1o.tensor, offset=ag1o.offset,
                      ap=[[6, 128], [1, 6], [768, NCORES]]))
            tot1 = stp.tile([128, 6], f32, tag="tot1")
            nc.vector.reduce_sum(tot1, g1, axis=AX.XY)
            mean1 = stp.tile([128, 3], f32, tag="mean1")
            nc.vector.tensor_scalar_mul(mean1, tot1[:, 0::2], 1.0 / CNT1)
            e2 = stp.tile([128, 3], f32, tag="e2")
            nc.vector.tensor_scalar(out=e2, in0=tot1[:, 1::2],
                                    scalar1=1.0 / CNT1, scalar2=EPS,
                                    op0=ALU.mult, op1=ALU.add)
            m2 = stp.tile([128, 3], f32, tag="m2")
            nc.vector.tensor_mul(m2, mean1, mean1)
            nc.vector.tensor_sub(e2, e2, m2)
            nc.scalar.activation(e2, e2, AF.Sqrt)
            nc.vector.reciprocal(e2, e2)
            nc.vector.tensor_mul(scale1, gx_sb, e2)
            nc.vector.tensor_mul(m2, mean1, scale1)
            nc.vector.tensor_sub(shift1, btx_sb, m2)
            nc.vector.reciprocal(inv1, scale1)
            nc.vector.tensor_scalar_mul(inv6, inv1, 6.0)
            nc.vector.tensor_mul(gi_zr, gzr_sb, inv1[:, 0:2])
            nc.vector.tensor_mul(btinv_zr, btzr_sb, inv1[:, 0:2])
            nc.vector.tensor_mul(gi_hh, ghh_sb, inv1[:, 2:3])
            nc.vector.tensor_mul(btinv_hh, bthh_sb, inv1[:, 2:3])

            # ================= Phase 2: recurrence =================
            with tc.tile_pool(name="crawp", bufs=3) as crawp, \
                 tc.tile_pool(name="wkp", bufs=3) as wkp, \
                 tc.tile_pool(name="zp", bufs=2) as zp:

                def emit_halo(step):
                    """Send h edge rows; consumed at the top of `step`."""
                    hbi = dram2.tile([128, 4, 64], f16, tag="hbi",
                                     name=f"hbi{step}")
                    nc.sync.dma_start(hbi[:, 0:2, :], h_pad[:, 2:4, 1:65])
                    nc.sync.dma_start(hbi[:, 2:4, :], h_pad[:, 32:34, 1:65])
                    hbo = dram2.tile([2, 128, 4, 64], f16, tag="hbo",
                                     name=f"hbo{step}")
                    ag(hbi.opt(), hbo.opt(), PAIRS)
                    return hbo

                # ---- t = 0: h0 = sigmoid(x_z) * tanh(x_h) ----
                z0 = zp.tile([128, OWN, 64], f16, tag="z")
                nc.scalar.activation(z0, y_sb[0][0][:, 1:33, :], AF.Sigmoid,
                                     scale=scale1[:, 0:1], bias=shift1[:, 0:1])
                th0 = wkp.tile([128, ROWS, 64], f16, tag="work")
                nc.scalar.activation(th0[:, 0:32, :], y_sb[0][2][:, 1:33, :],
                                     AF.Tanh,
                                     scale=scale1[:, 2:3], bias=shift1[:, 2:3])
                nc.vector.tensor_mul(h_pad[:, 2:34, 1:65], z0, th0[:, 0:32, :])
                hbo_prev = emit_halo(1)
                nc.sync.dma_start(outd[0], h_pad[:, 2:34, 1:65])

                for t in range(1, T):
                    # ---- land halo rows sent at the previous step's tail ----
                    nc.sync.dma_start(h_pad[:, 0:2, 1:65],
                                      hbo_prev[0, :, 2:4, :])
                    nc.vector.tensor_scalar_mul(h_pad[:, 0:2, 1:65],
                                                h_pad[:, 0:2, 1:65], mpre_sb)
                    nc.sync.dma_start(h_pad[:, 34:36, 1:65],
                                      hbo_prev[1, :, 0:2, :])
                    nc.vector.tensor_scalar_mul(h_pad[:, 34:36, 1:65],
                                                h_pad[:, 34:36, 1:65],
                                                mpost_sb)

                    # ---- conv_zr: interior strips first (halo-free) ----
                    st_zr = stp.tile([128, 2, 3], f32, tag="stzr")
                    craw = []
                    for c2 in range(2):
                        pg = ps.tile([128, 16, 64], f32, tag="ps")
                        for k in range(9):
                            ky, kx = k // 3, k % 3
                            wap = wzr_sb[:, c2 * 9 + k, :]
                            for si, i0 in enumerate((9, 17)):
                                nc.tensor.matmul(
                                    pg[:, 8 * si:8 * si + 8, :], wap,
                                    h_pad[:, i0 + ky:i0 + ky + 8, kx:kx + 64],
                                    start=(k == 0), stop=(k == 8))
                        cr = crawp.tile([128, ROWS, 64], f16, tag="craw")
                        craw.append(cr)
                        nc.scalar.activation(cr[:, 9:25, :], pg, AF.Copy,
                                             accum_out=st_zr[:, c2, 0:1])
                    # ---- halo-dependent strips ----
                    for c2 in range(2):
                        pg = ps.tile([128, 16, 64], f32, tag="ps")
                        for k in range(9):
                            ky, kx = k // 3, k % 3
                            wap = wzr_sb[:, c2 * 9 + k, :]
                            for si, i0 in enumerate((1, 25)):
                                nc.tensor.matmul(
                                    pg[:, 8 * si:8 * si + 8, :], wap,
                                    h_pad[:, i0 + ky:i0 + ky + 8, kx:kx + 64],
                                    start=(k == 0), stop=(k == 8))
                        nc.scalar.activation(craw[c2][:, 1:9, :],
                                             pg[:, 0:8, :], AF.Copy,
                                             accum_out=st_zr[:, c2, 1:2])
                        nc.scalar.activation(craw[c2][:, 25:33, :],
                                             pg[:, 8:16, :], AF.Copy,
                                             accum_out=st_zr[:, c2, 2:3])

                    # ---- zr BN stats (sumsq on DVE) + AllGather ----
                    stv = stp.tile([128, 2, 2], f32, tag="stv")
                    for c2 in range(2):
                        scr = scrp.tile([128, OWN, 64], f16, tag="scr")
                        nc.vector.tensor_mul(scr, craw[c2][:, 1:33, :],
                                             craw[c2][:, 1:33, :])
                        nc.vector.tensor_scalar(
                            out=scr, in0=scr, scalar1=1.0, scalar2=None,
                            op0=ALU.mult, op1=ALU.add,
                            accum_out=stv[:, c2, 1:2])
                        nc.vector.reduce_sum(stv[:, c2, 0:1],
                                             st_zr[:, c2, :], axis=AX.X)
                    sti = dram2.tile([128, 2, 2], f32, tag="sti")
                    nc.sync.dma_start(sti, stv)
                    sto = dram2.tile([NCORES, 128, 4], f32, tag="sto")
                    ag(sti.opt(), sto.opt(), ALL)

                    # edge rows for r (not in stats): overlap the AllGather
                    pse = psE.tile([128, 2, 64], f32, tag="pse", name="pse2")
                    for k in range(9):
                        ky, kx = k // 3, k % 3
                        nc.tensor.matmul(
                            pse, wzr_sb[:, 9 + k, :],
                            h_pad[:, ky::33, kx:kx + 64],
                            start=(k == 0), stop=(k == 8))
                    nc.scalar.activation(craw[1][:, 0::33, :], pse, AF.Copy)
                    warm(72)
                    preload([AF.Sqrt])
                    g2 = stp.tile([128, 4, NCORES], f32, tag="g2")
                    nc.sync.dma_start(
                        g2, _b.AP(tensor=sto.tensor, offset=sto.offset,
                                  ap=[[4, 128], [1, 4], [512, NCORES]]))
                    tot2 = stp.tile([128, 4], f32, tag="tot2")
                    nc.vector.reduce_sum(tot2, g2, axis=AX.X)
                    mean2 = stp.tile([128, 2], f32, tag="mean2")
                    nc.vector.tensor_scalar_mul(mean2, tot2[:, 0::2],
                                                1.0 / CNT2)
                    e22 = stp.tile([128, 2], f32, tag="e22")
                    nc.vector.tensor_scalar(out=e22, in0=tot2[:, 1::2],
                                            scalar1=1.0 / CNT2, scalar2=EPS,
                                            op0=ALU.mult, op1=ALU.add)
                    m22 = stp.tile([128, 2], f32, tag="m22")
                    nc.vector.tensor_mul(m22, mean2, mean2)
                    nc.vector.tensor_sub(e22, e22, m22)
                    nc.scalar.activation(e22, e22, AF.Sqrt)
                    preload([AF.Sigmoid])
                    nc.vector.reciprocal(e22, e22)
                    # sc2/sh2 have 1/s1 folded in so raw y adds directly
                    sc2 = stp.tile([128, 2], f32, tag="sc2")
                    nc.vector.tensor_mul(sc2, gi_zr, e22)
                    sh2 = stp.tile([128, 2], f32, tag="sh2")
                    nc.vector.tensor_mul(m22, mean2, sc2)
                    nc.vector.tensor_sub(sh2, btinv_zr, m22)

                    # ---- r chain (rows 0..33), split for conv_hh overlap ----
                    wk1 = wkp.tile([128, ROWS, 64], f16, tag="work")
                    for (a, b) in ((0, 26), (26, 34)):
                        nc.vector.tensor_scalar(
                            out=wk1[:, a:b, :], in0=craw[1][:, a:b, :],
                            scalar1=sc2[:, 1:2], scalar2=sh2[:, 1:2],
                            op0=ALU.mult, op1=ALU.add)
                        nc.vector.tensor_scalar(
                            out=wk1[:, a:b, :], in0=wk1[:, a:b, :],
                            scalar1=0.0, scalar2=inv6[:, 1:2],
                            op0=ALU.max, op1=ALU.min)
                        nc.vector.tensor_add(wk1[:, a:b, :], wk1[:, a:b, :],
                                             y_sb[t][1][:, a:b, :])
                        nc.scalar.activation(wk1[:, a:b, :], wk1[:, a:b, :],
                                             AF.Sigmoid,
                                             scale=scale1[:, 1:2],
                                             bias=shift1[:, 1:2])
                        nc.vector.tensor_mul(rh_pad[:, 1 + a:1 + b, 1:65],
                                             wk1[:, a:b, :],
                                             h_pad[:, 1 + a:1 + b, 1:65])

                    # ---- conv_hh group 0 (rh rows 1..26 only) ----
                    st_hh = stp.tile([128, 2], f32, tag="sthh")
                    ch = crawp.tile([128, ROWS, 64], f16, tag="craw")
                    hhpg = []
                    for g, (ja, jb) in enumerate(((0, 8), (16, 24))):
                        pg = ps.tile([128, 16, 64], f32, tag="ps")
                        hhpg.append(pg)
                        for k in range(9):
                            ky, kx = k // 3, k % 3
                            wap = whh_sb[:, k, :]
                            for si, j0 in enumerate((ja, jb)):
                                nc.tensor.matmul(
                                    pg[:, 8 * si:8 * si + 8, :], wap,
                                    rh_pad[:, j0 + 1 + ky:j0 + 1 + ky + 8,
                                           kx:kx + 64],
                                    start=(k == 0), stop=(k == 8))
                        nc.scalar.activation(
                            ch[:, 16 * g:16 * g + 16, :], pg, AF.Copy,
                            accum_out=st_hh[:, g:g + 1])

                    # ---- z chain (rows 1..32), overlaps conv_hh ----
                    wk0 = wkp.tile([128, ROWS, 64], f16, tag="work")
                    nc.vector.tensor_scalar(
                        out=wk0[:, 0:32, :], in0=craw[0][:, 1:33, :],
                        scalar1=sc2[:, 0:1], scalar2=sh2[:, 0:1],
                        op0=ALU.mult, op1=ALU.add)
                    nc.vector.tensor_scalar(
                        out=wk0[:, 0:32, :], in0=wk0[:, 0:32, :], scalar1=0.0,
                        scalar2=inv6[:, 0:1], op0=ALU.max, op1=ALU.min)
                    nc.vector.tensor_add(wk0[:, 0:32, :], wk0[:, 0:32, :],
                                         y_sb[t][0][:, 1:33, :])
                    z = zp.tile([128, OWN, 64], f16, tag="z")
                    nc.scalar.activation(z, wk0[:, 0:32, :], AF.Sigmoid,
                                         scale=scale1[:, 0:1],
                                         bias=shift1[:, 0:1])

                    # ---- hh BN stats (sumsq on DVE) + AllGather ----
                    stv2 = stp.tile([128, 2], f32, tag="stv2")
                    scr = scrp.tile([128, OWN, 64], f16, tag="scr")
                    nc.vector.tensor_mul(scr, ch[:, 0:32, :], ch[:, 0:32, :])
                    nc.vector.tensor_scalar(
                        out=scr, in0=scr, scalar1=1.0, scalar2=None,
                        op0=ALU.mult, op1=ALU.add, accum_out=stv2[:, 1:2])
                    nc.vector.reduce_sum(stv2[:, 0:1], st_hh, axis=AX.X)
                    sti2 = dram2.tile([128, 2], f32, tag="sti2")
                    nc.sync.dma_start(sti2, stv2)
                    sto2 = dram2.tile([NCORES, 128, 2], f32, tag="sto2")
                    ag(sti2.opt(), sto2.opt(), ALL)
                    warm(88)
                    preload([AF.Sqrt])
                    g3 = stp.tile([128, 2, NCORES], f32, tag="g3")
                    nc.sync.dma_start(
                        g3, _b.AP(tensor=sto2.tensor, offset=sto2.offset,
                                  ap=[[2, 128], [1, 2], [256, NCORES]]))
                    tot3 = stp.tile([128, 2], f32, tag="tot3")
                    nc.vector.reduce_sum(tot3, g3, axis=AX.X)
                    mean3 = stp.tile([128, 1], f32, tag="mean3")
                    nc.vector.tensor_scalar_mul(mean3, tot3[:, 0:1],
                                                1.0 / CNT2)
                    e23 = stp.tile([128, 1], f32, tag="e23")
                    nc.vector.tensor_scalar(out=e23, in0=tot3[:, 1:2],
                                            scalar1=1.0 / CNT2, scalar2=EPS,
                                            op0=ALU.mult, op1=ALU.add)
                    m23 = stp.tile([128, 1], f32, tag="m23")
                    nc.vector.tensor_mul(m23, mean3, mean3)
                    nc.vector.tensor_sub(e23, e23, m23)
                    nc.scalar.activation(e23, e23, AF.Sqrt)
                    preload([AF.Tanh])
                    nc.vector.reciprocal(e23, e23)
                    sc3 = stp.tile([128, 1], f32, tag="sc3")
                    nc.vector.tensor_mul(sc3, gi_hh, e23)
                    sh3 = stp.tile([128, 1], f32, tag="sh3")
                    nc.vector.tensor_mul(m23, mean3, sc3)
                    nc.vector.tensor_sub(sh3, btinv_hh, m23)

                    # ---- hh chain -> tanh -> update h. Edge rows first so
                    # the next step's halo AllGather fires early; bulk rows
                    # last gate the next step's interior conv ----
                    wkh = wkp.tile([128, ROWS, 64], f16, tag="work")

                    def upd(a, b):
                        nc.vector.tensor_scalar(
                            out=wkh[:, a:b, :], in0=ch[:, a:b, :],
                            scalar1=sc3, scalar2=sh3,
                            op0=ALU.mult, op1=ALU.add)
                        nc.vector.tensor_scalar(
                            out=wkh[:, a:b, :], in0=wkh[:, a:b, :],
                            scalar1=0.0, scalar2=inv6[:, 2:3],
                            op0=ALU.max, op1=ALU.min)
                        nc.vector.tensor_add(wkh[:, a:b, :], wkh[:, a:b, :],
                                             y_sb[t][2][:, 1 + a:1 + b, :])
                        nc.scalar.activation(wkh[:, a:b, :], wkh[:, a:b, :],
                                             AF.Tanh, scale=scale1[:, 2:3],
                                             bias=shift1[:, 2:3])
                        nc.vector.tensor_sub(wkh[:, a:b, :], wkh[:, a:b, :],
                                             h_pad[:, 2 + a:2 + b, 1:65])
                        nc.vector.tensor_mul(wkh[:, a:b, :], z[:, a:b, :],
                                             wkh[:, a:b, :])
                        nc.vector.tensor_add(h_pad[:, 2 + a:2 + b, 1:65],
                                             h_pad[:, 2 + a:2 + b, 1:65],
                                             wkh[:, a:b, :])

                    upd(0, 3)
                    upd(29, 32)
                    if t < T - 1:
                        hbo_prev = emit_halo(t + 1)
                    upd(3, 29)
                    nc.sync.dma_start(outd[t], h_pad[:, 2:34, 1:65])

    nc.compile()
    return nc


def get_prog():
    global _PROG
    if _PROG is None:
        _PROG = _build_program()
    return _PROG


def prep_in_maps(x, w_x2h, g_x2h, bt_x2h, w_zr, g_zr, bt_zr, w_hh, g_hh,
                 bt_hh):
    """Shard + pre-transform inputs on the host. Returns per-core dicts."""
    x = np.asarray(x, np.float32)
    w_x2h = np.asarray(w_x2h, np.float32)
    w_zr = np.asarray(w_zr, np.float32)
    w_hh = np.asarray(w_hh, np.float32)
    if MM_DT == "float16":
        f16 = np.float16
    elif MM_DT == "bfloat16":
        import ml_dtypes
        f16 = ml_dtypes.bfloat16
    else:
        f16 = np.float32

    # 5x5 weights: column pairs (wp), row pairs on kx=4 (wq), single (ws)
    wp = np.zeros((128, 30, 128), f16)
    wq = np.zeros((128, 6, 128), f16)
    ws = np.zeros((64, 3, 128), f16)
    for c3 in range(3):
        cs = slice(128 * c3, 128 * (c3 + 1))
        for ky in range(5):
            for e in range(2):
                wp[0:64, c3 * 10 + ky * 2 + e] = w_x2h[cs, :, ky, 2 * e].T
                wp[64:128, c3 * 10 + ky * 2 + e] = \
                    w_x2h[cs, :, ky, 2 * e + 1].T
        for p in range(2):
            wq[0:64, c3 * 2 + p] = w_x2h[cs, :, 2 * p, 4].T
            wq[64:128, c3 * 2 + p] = w_x2h[cs, :, 2 * p + 1, 4].T
        ws[:, c3] = w_x2h[cs, :, 4, 4].T
    wzr = np.zeros((128, 18, 128), f16)
    for c2 in range(2):
        cs = slice(128 * c2, 128 * (c2 + 1))
        for ky in range(3):
            for kx in range(3):
                wzr[:, c2 * 9 + ky * 3 + kx] = w_zr[cs, :, ky, kx].T
    whh = np.zeros((128, 9, 128), f16)
    for ky in range(3):
        for kx in range(3):
            whh[:, ky * 3 + kx] = w_hh[:, :, ky, kx].T

    gx = np.ascontiguousarray(np.asarray(g_x2h, np.float32).reshape(3, 128).T)
    btx = np.ascontiguousarray(
        np.asarray(bt_x2h, np.float32).reshape(3, 128).T)
    gzr = np.ascontiguousarray(np.asarray(g_zr, np.float32).reshape(2, 128).T)
    btzr = np.ascontiguousarray(
        np.asarray(bt_zr, np.float32).reshape(2, 128).T)
    ghh = np.asarray(g_hh, np.float32).reshape(128, 1)
    bthh = np.asarray(bt_hh, np.float32).reshape(128, 1)

    shared = dict(wp=wp, wq=wq, ws=ws, wzr=wzr, whh=whh, gx=gx, btx=btx,
                  gzr=gzr, btzr=btzr, ghh=ghh, bthh=bthh)

    in_maps = []
    for c in range(NCORES):
        n, half = c // 2, c % 2
        base = 32 * half
        # padded input band: rows base-3..base+34, cols -2..65
        xpad = np.zeros((T, CIN, HP1, WP1), np.float32)
        i0 = max(0, base - 3)
        i1 = min(H, base + 35)
        j0 = i0 - (base - 3)
        xpad[:, :, j0:j0 + (i1 - i0), 2:66] = x[:, n, :, i0:i1, :]
        x2 = np.zeros((T, 128, HP1, WP1), f16)
        x2[:, 0:64] = xpad
        x2[:, 64:128, :, 0:WP1 - 1] = xpad[:, :, :, 1:WP1]
        x3 = np.zeros((T, 128, HP1, 64), f16)
        x3[:, 0:64] = xpad[:, :, :, 4:68]
        x3[:, 64:128, 0:HP1 - 1, :] = xpad[:, :, 1:HP1, 4:68]
        m = dict(shared)
        m["x2"] = x2
        m["x3"] = x3
        m["mpre"] = np.full((128, 1), 0.0 if half == 0 else 1.0, np.float32)
        m["mpost"] = np.full((128, 1), 1.0 if half == 0 else 0.0, np.float32)
        in_maps.append(m)
    return in_maps


def assemble_output(results):
    out = np.zeros((T, N, CD, H, W), np.float32)
    for c in range(NCORES):
        n, half = c // 2, c % 2
        out[:, n, :, 32 * half:32 * half + 32, :] = \
            results[c]["out"].astype(np.float32)
    return out


def kernel(**inputs):
    from concourse import bass_utils
    nc = get_prog()
    in_maps = prep_in_maps(
        inputs["x"], inputs["w_x2h"], inputs["g_x2h"], inputs["bt_x2h"],
        inputs["w_zr"], inputs["g_zr"], inputs["bt_zr"],
        inputs["w_hh"], inputs["g_hh"], inputs["bt_hh"])
    res = bass_utils.run_bass_kernel_spmd(nc, in_maps,
                                          core_ids=list(range(NCORES)))
    return assemble_output(res.results)
